# revision 24
# baseline (speedup 1.0000x reference)
"""Trainium2 Bass kernel for KernelAttention (gaussian-kernel multi-head attention).

Math (per batch b):
  d2[q,k]   = |q_pos[q] - k_pos[k]|^2   (computed as m = -d2 via one K=5 augmented matmul)
  s_h[k,q]  = exp(-c_h * d2),  c_h = 1/lengthscale_h^2   (masked keys contribute 0)
  att_h[q,v]= sum_k s_h[k,q] * V[k,h,v] / (sum_k s_h[k,q]*unmasked[k] + 1e-5)
  out[q,o]  = sum_{h,v} att_h[q,v] * w_out[o, h*64+v]

Sharding: 8 cores = (batch b in 0..3) x (query half in 0..1); each core owns
[1024 q, 2048 k]. All inputs host-prepped per core; outputs gathered on host.

Head strategy (c = [100, 25, 4, 1, 0.25, 0.04, 0.01, 0.0025]):
  * heads 5,6,7 (ls=5,10,20): LOW-RANK. exp(-c d2) = g(q) f(k) exp(2c q.k) and
    exp(2c q.k) is a deg<=7 polynomial in q.k => scores factor through <=120
    monomial features psi(k), phi(q). att_h = (psi f V)^T phi' costs O(L*r)
    matmuls; the O(L^2) scores never materialize.
  * heads 1,4 (c=25, 0.25): ACT exp straight from the d2 PSUM per ktile.
  * head 3 (c=1): ACT exp for 3 of 4 k-groups, DVE chain s3=s4^4 for the rest
    (ACT/DVE balance + precision knob).
  * heads 0,2: DVE squaring chains s0=s1^4, s2=s3^4 (bf16, in-place x^2 twice).
PSUM (8 banks) is the binding constraint: tag "d2" = 2 rotating [128,1024]
dist tiles; tag "att" = 2 slots shared in sequence by the lowrank M, lowrank
atts, the two in-window accumulators (heads 1,4), rb broadcasts and po tiles.
Heads 3,0,2 attend post-window in the freed slots. Normalization: ones-column
makes att row 64 the masked score sum; r=1/(n+eps) via ACT Ln/Exp per pair
batch; sel8 matmul broadcasts r (hi/lo bf16) to 128 partitions; DVE applies
it reading the rb PSUM directly.
"""

import math
import numpy as np
from contextlib import ExitStack

B, LQ, LK, DPOS = 4, 2048, 2048, 3
H, V, OUTD = 8, 64, 512
QS = LQ // 2          # q rows per core
KT = LK // 128        # k tiles
NG, GK = 4, KT // 4   # k groups of 4 ktiles
V1 = V + 1            # value cols + ones col
NCORES = 8

MAXDEG = 7
DEG = {5: 7, 6: 5, 7: 4}           # Taylor degree per low-rank head
ACT3_GROUPS = (0, 1, 2)            # k-groups where s3 comes from ACT exp
KGORDER = (3, 0, 1, 2)             # k-group processing order (chain group first)

def monomial_list(maxdeg):
    return [(a, b, d - a - b) for d in range(maxdeg + 1)
            for a in range(d + 1) for b in range(d - a + 1)]

MONS = monomial_list(MAXDEG)
RMAX = len(MONS)                   # 120
RH = {h: len(monomial_list(DEG[h])) for h in (5, 6, 7)}

_cache = {}


def _build(cv):
    key = tuple(cv)
    if key in _cache:
        return _cache[key]
    import concourse.bacc as bacc
    import concourse.tile as tile
    from concourse import mybir

    f32 = mybir.dt.float32
    bf16 = mybir.dt.bfloat16
    AF = mybir.ActivationFunctionType

    nc = bacc.Bacc("TRN2", target_bir_lowering=False, debug=False,
                   num_devices=NCORES)
    ka = nc.dram_tensor("ka", [32, LK], bf16, kind="ExternalInput").ap()
    qa = nc.dram_tensor("qa", [32, QS], bf16, kind="ExternalInput").ap()
    vp = nc.dram_tensor("vp", [128, KT, 5 * V1], bf16, kind="ExternalInput").ap()
    vpf = nc.dram_tensor("vpf", [128, KT, 3 * V1], bf16, kind="ExternalInput").ap()
    psi = nc.dram_tensor("psi", [128, KT, RMAX], bf16, kind="ExternalInput").ap()
    phi = nc.dram_tensor("phi", [RMAX, 3, QS], bf16, kind="ExternalInput").ap()
    wt = nc.dram_tensor("wt", [128, 4, OUTD], bf16, kind="ExternalInput").ap()
    sel8 = nc.dram_tensor("sel8", [8, 4, 128], bf16, kind="ExternalInput").ap()
    outT = nc.dram_tensor("outT", [OUTD, QS], f32, kind="ExternalOutput").ap()

    with tile.TileContext(nc) as tc, ExitStack() as ctx:
        const = ctx.enter_context(tc.tile_pool(name="const", bufs=1))
        spool = ctx.enter_context(tc.tile_pool(name="spool", bufs=16))
        stage = ctx.enter_context(tc.tile_pool(name="stage", bufs=2))
        obuf = ctx.enter_context(tc.tile_pool(name="obuf", bufs=2))
        psp = ctx.enter_context(tc.tile_pool(name="psum", bufs=2, space="PSUM"))

        # inputs on parallel DMA queues: dist operands (sync) land first so
        # the d2 pipeline starts ~immediately; lowrank tables and vp on other
        # queues in parallel.
        # rows 0:15 carry the hi/lo operands, 15:31 host-zeroed; rows 32:128
        # zeroed on-device so only 32 rows transfer but the PE contracts K=128
        # (keeps the HAM activity monitor seeing a dense array).
        ka_sb = const.tile([128, LK], bf16)
        qa_sb = const.tile([128, QS], bf16)
        for p0 in (32, 64, 96):   # engines: max 32 partitions off-base-0
            nc.vector.memset(ka_sb[p0:p0 + 32, :], 0.0)
            nc.vector.memset(qa_sb[p0:p0 + 32, :], 0.0)
        nc.gpsimd.dma_start(out=ka_sb[0:32, :], in_=ka)
        nc.gpsimd.dma_start(out=qa_sb[0:32, :], in_=qa)
        psi_sb = const.tile([128, KT, RMAX], bf16)
        nc.gpsimd.dma_start(out=psi_sb[:], in_=psi)
        vpf_sb = const.tile([128, KT, 3 * V1], bf16)
        nc.gpsimd.dma_start(out=vpf_sb[:], in_=vpf)
        vp_sb = const.tile([128, KT, 5 * V1], bf16)
        nc.scalar.dma_start(out=vp_sb[:], in_=vp)
        phi_sb = const.tile([RMAX, 3, QS], bf16)
        nc.gpsimd.dma_start(out=phi_sb[:], in_=phi)
        wt_sb = const.tile([128, 4, OUTD], bf16)
        nc.sync.dma_start(out=wt_sb[:], in_=wt)
        sel8_sb = const.tile([8, 4, 128], bf16)
        nc.sync.dma_start(out=sel8_sb[:], in_=sel8)

        flat = [const.tile([128, QS], bf16, tag=f"flat{j}", name=f"flat{j}")
                for j in range(4)]
        norms = const.tile([8, QS], f32)
        nc.vector.memset(norms[:], 1.0)
        eps_t = const.tile([8, 1], f32)
        nc.vector.memset(eps_t[:], 1e-5)
        r_all = const.tile([8, QS], f32)
        r_hi = const.tile([8, QS], bf16)
        nc.vector.memset(r_hi[:], 0.0)
        r_lo = const.tile([8, QS], bf16)
        nc.vector.memset(r_lo[:], 0.0)
        Mb = const.tile([RMAX, 3 * V1], bf16)

        kt_order = [g * GK + i for g in KGORDER for i in range(GK)]

        # ---- PE: first dist ktiles lead (ACT window starts ASAP), then the
        # lowrank M matmuls fill while the d2 rotation cycles. ----
        kt_lead = kt_order[:2]

        def dist_mms(kt, d2):
            for qc in range(2):
                s5 = slice(qc * 512, (qc + 1) * 512)
                nc.tensor.matmul(d2[:, s5],
                                 lhsT=ka_sb[:, kt * 128:(kt + 1) * 128],
                                 rhs=qa_sb[:, s5], start=True, stop=True)

        d2_lead = {}
        for kt in kt_lead:
            d2_lead[kt] = psp.tile([128, QS], f32, tag="d2", name=f"d2_{kt}")
            dist_mms(kt, d2_lead[kt])
        m_ps = psp.tile([RMAX, 3 * V1], f32, tag="att", name="m_ps")
        for i in range(KT):
            nc.tensor.matmul(m_ps[:], lhsT=psi_sb[:, i, :],
                             rhs=vpf_sb[:, i, :],
                             start=(i == 0), stop=(i == KT - 1))
        nc.vector.tensor_copy(out=Mb[:], in_=m_ps[:])

        latt = {}
        for h in (5, 6, 7):
            a = psp.tile([V1, QS], f32, tag="att", name=f"latt{h}")
            r = RH[h]
            for qc in range(2):
                s5 = slice(qc * 512, (qc + 1) * 512)
                nc.tensor.matmul(a[:, s5],
                                 lhsT=Mb[0:r, (h - 5) * V1:(h - 4) * V1],
                                 rhs=phi_sb[0:r, h - 5, s5],
                                 start=True, stop=True)
            latt[h] = a
        def norm_row(h, a, eng):
            # engines cannot cross to partition h directly (32-align rule):
            # same-partition copy to a stage row, then DMA moves partitions.
            stg = stage.tile([V1, QS], f32, tag="stg", name=f"stg{h}")
            if eng == "v":
                nc.vector.tensor_copy(out=stg[64:65, :], in_=a[64:65, :])
            else:
                nc.scalar.copy(out=stg[64:65, :], in_=a[64:65, :])
            nc.sync.dma_start(out=norms[h:h + 1, :], in_=stg[64:65, :])

        for h in (5, 6, 7):
            j, r0 = h // 2, (h % 2) * 64
            nc.vector.tensor_copy(out=flat[j][r0:r0 + 64, :],
                                  in_=latt[h][0:64, :])
            norm_row(h, latt[h], "v")

        # ---- score tiles ----
        # 20 logical tiles through 16 slots: allocation order pairs the
        # earliest-dying tiles (s1/s4 of the first-processed groups, consumed
        # by in-window attends + chains) with the latest-written ones
        # (idx i reuses slot of idx i-16), so slot reuse never stalls.
        g3, g0, g1, g2 = KGORDER
        s_order = [(1, g3), (4, g3), (1, g0), (4, g0),
                   (1, g1), (1, g2), (4, g1), (4, g2),
                   (3, g3), (3, g0), (3, g1), (3, g2),
                   (0, g3), (0, g0), (2, g3), (2, g0),
                   (2, g2), (0, g2), (2, g1), (0, g1)]
        s_t = {}
        for h, g in s_order:
            s_t[(h, g)] = spool.tile([128, GK, QS], bf16, tag="s",
                                     name=f"s{h}_{g}")

        # ---- in-window accumulators (heads 1 and 4) ----
        att1 = psp.tile([V1, QS], f32, tag="att", name="att1")
        att4 = psp.tile([V1, QS], f32, tag="att", name="att4")

        def attend(h, a, kt, i):
            for qc in range(2):
                s5 = slice(qc * 512, (qc + 1) * 512)
                nc.tensor.matmul(a[:, s5],
                                 lhsT=vp_sb[:, kt, h * V1:(h + 1) * V1],
                                 rhs=s_t[(h, kt // GK)][:, kt % GK, s5],
                                 start=(i == 0), stop=(i == KT - 1))

        # ---- dist + ACT exps + in-window attends, ktile-pipelined ----
        for i, kt in enumerate(kt_order):
            g, kk = kt // GK, kt % GK
            if kt in d2_lead:
                d2 = d2_lead[kt]
            else:
                d2 = psp.tile([128, QS], f32, tag="d2", name=f"d2_{kt}")
                dist_mms(kt, d2)
            nc.scalar.activation(out=s_t[(1, g)][:, kk, :], in_=d2[:],
                                 func=AF.Exp, scale=float(cv[1]))
            nc.scalar.activation(out=s_t[(4, g)][:, kk, :], in_=d2[:],
                                 func=AF.Exp, scale=float(cv[4]))
            if g in ACT3_GROUPS:
                nc.scalar.activation(out=s_t[(3, g)][:, kk, :], in_=d2[:],
                                     func=AF.Exp, scale=float(cv[3]))
            if i >= 2:
                kp = kt_order[i - 2]
                attend(1, att1, kp, i - 2)
                attend(4, att4, kp, i - 2)
        for i in (KT - 2, KT - 1):
            attend(1, att1, kt_order[i], i)
            attend(4, att4, kt_order[i], i)

        # ---- DVE chains, fine-grained (2-ktile slices chase the ACT stream).
        # 2nd mul is in-place (1x DVE mode) -- chains have slack vs the window.
        def chain(dh, sh, g, kk):
            s2k = (slice(None), slice(kk, kk + 2), slice(None))
            nc.vector.tensor_mul(s_t[(dh, g)][s2k], s_t[(sh, g)][s2k],
                                 s_t[(sh, g)][s2k])
            nc.vector.tensor_mul(s_t[(dh, g)][s2k], s_t[(dh, g)][s2k],
                                 s_t[(dh, g)][s2k])

        g3 = KGORDER[0]
        for kk in (0, 2):
            chain(0, 1, g3, kk)
            chain(3, 4, g3, kk)
            chain(2, 3, g3, kk)
        for g in KGORDER[1:]:
            for kk in (0, 2):
                chain(0, 1, g, kk)
                chain(2, 3, g, kk)

        # ---- post-window attends in freed slots (contiguous bursts) ----
        def recip(h0, h1):
            # full-range [0:8] ops (non-32-aligned partition slices are
            # illegal); recomputing other pairs' rows is idempotent and
            # sel8 masks them in the rb matmuls anyway.
            nc.scalar.activation(out=r_all[:], in_=norms[:],
                                 func=AF.Ln, bias=eps_t[:])
            nc.scalar.activation(out=r_all[:], in_=r_all[:],
                                 func=AF.Exp, scale=-1.0)
            nc.vector.tensor_copy(out=r_hi[:], in_=r_all[:])
            nc.vector.tensor_sub(r_lo[:], r_all[:], r_hi[:])

        def rb_mms(rb, j):
            for qc in range(2):
                s5 = slice(qc * 512, (qc + 1) * 512)
                nc.tensor.matmul(rb[:, s5], lhsT=sel8_sb[:, j, :],
                                 rhs=r_hi[:, s5], start=True, stop=False)
                nc.tensor.matmul(rb[:, s5], lhsT=sel8_sb[:, j, :],
                                 rhs=r_lo[:, s5], start=False, stop=True)

        def evac(h, a):
            j, r0 = h // 2, (h % 2) * 64
            nc.vector.tensor_copy(out=flat[j][r0:r0 + 64, :], in_=a[0:64, :])

        def normalize(h, rb):
            j, r0 = h // 2, (h % 2) * 64
            nc.vector.tensor_mul(flat[j][r0:r0 + 64, :],
                                 flat[j][r0:r0 + 64, :], rb[r0:r0 + 64, :])

        # post-window burst order [att2, att3, att0]: the pair-(2,3)
        # normalization ladder then hides under att0's burst, leaving only
        # the (0,1) ladder exposed at the tail.
        att2 = psp.tile([V1, QS], f32, tag="d2", name="att2")
        for i, kt in enumerate(kt_order):
            attend(2, att2, kt, i)
        att3 = psp.tile([V1, QS], f32, tag="d2", name="att3")
        for i, kt in enumerate(kt_order):
            attend(3, att3, kt, i)
        att0 = psp.tile([V1, QS], f32, tag="d2", name="att0")

        # head 1/4 wrap-up (window end)
        norm_row(1, att1, "v")
        evac(1, att1)
        norm_row(4, att4, "v")
        evac(4, att4)

        po = {}

        def po_mms(ot, j, start, stop):
            for qc in range(2):
                s5 = slice(qc * 512, (qc + 1) * 512)
                nc.tensor.matmul(po[ot][:, s5],
                                 lhsT=wt_sb[:, j, ot * 128:(ot + 1) * 128],
                                 rhs=flat[j][:, s5], start=start, stop=stop)

        norm_row(2, att2, "v")
        evac(2, att2)

        # att0 burst
        for i, kt in enumerate(kt_order):
            attend(0, att0, kt, i)

        norm_row(3, att3, "v")
        evac(3, att3)
        recip(2, 8)          # covers pairs (2,3),(4,5),(6,7) under att0

        rb23 = psp.tile([128, QS], f32, tag="att", name="rb23")
        rb_mms(rb23, 1)
        normalize(3, rb23)
        normalize(2, rb23)
        rb45 = psp.tile([128, QS], f32, tag="att", name="rb45")
        rb_mms(rb45, 2)
        normalize(4, rb45)
        normalize(5, rb45)
        rb67 = psp.tile([128, QS], f32, tag="att", name="rb67")
        rb_mms(rb67, 3)
        normalize(6, rb67)
        normalize(7, rb67)

        po[2] = psp.tile([128, QS], f32, tag="d2", name="po2")
        for ji, j in enumerate((1, 2, 3)):
            po_mms(2, j, ji == 0, False)
        po[3] = psp.tile([128, QS], f32, tag="att", name="po3")
        for ji, j in enumerate((1, 2, 3)):
            po_mms(3, j, ji == 0, False)
        po[0] = psp.tile([128, QS], f32, tag="att", name="po0")
        for ji, j in enumerate((1, 2, 3)):
            po_mms(0, j, ji == 0, False)

        # tail ladder for pair (0,1): norm first (it gates everything)
        norm_row(0, att0, "v")
        evac(0, att0)
        recip(0, 2)
        rb01 = psp.tile([128, QS], f32, tag="d2", name="rb01")
        rb_mms(rb01, 0)
        normalize(0, rb01)
        normalize(1, rb01)
        po[1] = psp.tile([128, QS], f32, tag="att", name="po1")
        for ji, j in enumerate((1, 2, 3)):
            po_mms(1, j, ji == 0, False)
        po_mms(2, 0, False, True)
        po_mms(3, 0, False, True)
        po_mms(0, 0, False, True)
        po_mms(1, 0, False, True)

        oq = {0: nc.scalar, 1: nc.gpsimd, 2: nc.sync, 3: nc.gpsimd}
        for ot in range(4):
            ob = obuf.tile([128, QS], f32, tag="ob", name=f"ob{ot}")
            if ot % 2 == 0:
                nc.scalar.copy(out=ob[:], in_=po[ot][:])
            else:
                nc.vector.tensor_copy(out=ob[:], in_=po[ot][:])
            oq[ot].dma_start(out=outT[ot * 128:(ot + 1) * 128, :], in_=ob[:])

    nc.compile()
    _cache[key] = nc
    return nc


def _hilo(x, bf16):
    hi = x.astype(bf16)
    lo = (x - hi.astype(np.float32)).astype(bf16)
    return hi, lo


def _prep_core(qp, kp, vals, mask, w_out, cv, bf16):
    q2 = (qp * qp).sum(-1)
    one_q = np.ones(QS, np.float32)
    qa5 = np.stack([2 * qp[:, 0], 2 * qp[:, 1], 2 * qp[:, 2], -one_q, -q2]) \
        .astype(np.float32)
    k2 = (kp * kp).sum(-1)
    one_k = np.ones(LK, np.float32)
    ka5 = np.stack([kp[:, 0], kp[:, 1], kp[:, 2], k2, one_k]).astype(np.float32)
    ka_hi, ka_lo = _hilo(ka5, bf16)
    qa_hi, qa_lo = _hilo(qa5, bf16)
    # padded to 128 contraction rows (zeros) so the PE array reads as fully
    # active to the HAM clock gate during the dist matmuls.
    ka = np.zeros((32, LK), np.float32).astype(bf16)
    ka[0:5], ka[5:10], ka[10:15] = ka_hi, ka_lo, ka_hi
    qa = np.zeros((32, QS), np.float32).astype(bf16)
    qa[0:5], qa[5:10], qa[10:15] = qa_hi, qa_hi, qa_lo

    vv = np.concatenate([vals, np.ones((LK, H, 1), np.float32)], axis=-1)
    vv = vv.copy()
    vv[mask] = 0.0                               # [LK, H, 65]
    vp = vv[:, 0:5, :].reshape(LK, 5 * V1)
    vp = vp.reshape(KT, 128, 5 * V1).transpose(1, 0, 2).astype(bf16)

    vpf = np.empty((LK, 3, V1), np.float32)
    phi = np.zeros((RMAX, 3, QS), np.float32)
    for h in (5, 6, 7):
        c = float(cv[h])
        f = np.exp(-c * k2)
        vpf[:, h - 5, :] = vv[:, h, :] * f[:, None]
        g = np.exp(-c * q2)
        mons = monomial_list(DEG[h])
        coef = np.array([(2 * c) ** (a + b + cc) /
                         (math.factorial(a) * math.factorial(b) * math.factorial(cc))
                         for (a, b, cc) in mons], np.float32)
        ph = np.stack([qp[:, 0] ** a * qp[:, 1] ** b * qp[:, 2] ** cc
                       for (a, b, cc) in mons], axis=0) * coef[:, None]
        phi[0:len(mons), h - 5, :] = ph * g[None, :]
    vpf = vpf.reshape(KT, 128, 3 * V1).transpose(1, 0, 2).astype(bf16)
    psi = np.stack([kp[:, 0] ** a * kp[:, 1] ** b * kp[:, 2] ** cc
                    for (a, b, cc) in MONS], axis=1)        # [LK, RMAX]
    psi = psi.reshape(KT, 128, RMAX).transpose(1, 0, 2).astype(bf16)

    wt = np.ascontiguousarray(w_out.T).reshape(4, 128, OUTD) \
        .transpose(1, 0, 2).astype(bf16)
    sel8 = np.zeros((8, 4, 128), np.float32)
    for j in range(4):
        sel8[2 * j, j, :64] = 1.0
        sel8[2 * j + 1, j, 64:] = 1.0
    return {"ka": np.ascontiguousarray(ka), "qa": np.ascontiguousarray(qa),
            "vp": np.ascontiguousarray(vp), "vpf": np.ascontiguousarray(vpf),
            "psi": np.ascontiguousarray(psi), "phi": phi.astype(bf16),
            "wt": np.ascontiguousarray(wt), "sel8": sel8.astype(bf16)}


def kernel(query_positions, key_positions, values, masked_elements,
           lengthscales, w_out, _want_trace=False):
    import ml_dtypes
    from concourse.bass_utils import run_bass_kernel_spmd

    bf16 = ml_dtypes.bfloat16
    qp = np.asarray(query_positions, np.float32)
    kp = np.asarray(key_positions, np.float32)
    vals = np.asarray(values, np.float32)
    mask = np.asarray(masked_elements).astype(bool)
    ls = np.asarray(lengthscales, np.float32)
    w = np.asarray(w_out, np.float32)

    cv = (1.0 / (ls.astype(np.float64) ** 2)).astype(np.float32)
    nc = _build(tuple(float(x) for x in cv))

    in_maps = []
    for c in range(NCORES):
        b, hf = c // 2, c % 2
        in_maps.append(_prep_core(qp[b, hf * QS:(hf + 1) * QS], kp[b],
                                  vals[b], mask[b], w, cv, bf16))
    res = run_bass_kernel_spmd(nc, in_maps, core_ids=list(range(NCORES)),
                               trace=_want_trace)
    out = np.empty((B, LQ, OUTD), np.float32)
    for c in range(NCORES):
        b, hf = c // 2, c % 2
        out[b, hf * QS:(hf + 1) * QS, :] = res.results[c]["outT"].T
    if _want_trace:
        return out, res
    return out


# revision 25
# speedup vs baseline: 1.0274x; 1.0274x over previous
"""Trainium2 Bass kernel for KernelAttention (gaussian-kernel multi-head attention).

Math (per batch b):
  d2[q,k]   = |q_pos[q] - k_pos[k]|^2   (computed as m = -d2 via one K=5 augmented matmul)
  s_h[k,q]  = exp(-c_h * d2),  c_h = 1/lengthscale_h^2   (masked keys contribute 0)
  att_h[q,v]= sum_k s_h[k,q] * V[k,h,v] / (sum_k s_h[k,q]*unmasked[k] + 1e-5)
  out[q,o]  = sum_{h,v} att_h[q,v] * w_out[o, h*64+v]

Sharding: 8 cores = (batch b in 0..3) x (query half in 0..1); each core owns
[1024 q, 2048 k]. All inputs host-prepped per core; outputs gathered on host.

Head strategy (c = [100, 25, 4, 1, 0.25, 0.04, 0.01, 0.0025]):
  * heads 5,6,7 (ls=5,10,20): LOW-RANK. exp(-c d2) = g(q) f(k) exp(2c q.k) and
    exp(2c q.k) is a deg<=7 polynomial in q.k => scores factor through <=120
    monomial features psi(k), phi(q). att_h = (psi f V)^T phi' costs O(L*r)
    matmuls; the O(L^2) scores never materialize.
  * heads 1,4 (c=25, 0.25): ACT exp straight from the d2 PSUM per ktile.
  * head 3 (c=1): ACT exp for 3 of 4 k-groups, DVE chain s3=s4^4 for the rest
    (ACT/DVE balance + precision knob).
  * heads 0,2: DVE squaring chains s0=s1^4, s2=s3^4 (bf16, in-place x^2 twice).
PSUM (8 banks) is the binding constraint: tag "d2" = 2 rotating [128,1024]
dist tiles; tag "att" = 2 slots shared in sequence by the lowrank M, lowrank
atts, the two in-window accumulators (heads 1,4), rb broadcasts and po tiles.
Heads 3,0,2 attend post-window in the freed slots. Normalization: ones-column
makes att row 64 the masked score sum; r=1/(n+eps) via ACT Ln/Exp per pair
batch; sel8 matmul broadcasts r (hi/lo bf16) to 128 partitions; DVE applies
it reading the rb PSUM directly.
"""

import math
import numpy as np
from contextlib import ExitStack

B, LQ, LK, DPOS = 4, 2048, 2048, 3
H, V, OUTD = 8, 64, 512
QS = LQ // 2          # q rows per core
KT = LK // 128        # k tiles
NG, GK = 4, KT // 4   # k groups of 4 ktiles
V1 = V + 1            # value cols + ones col
NCORES = 8

MAXDEG = 7
DEG = {5: 7, 6: 5, 7: 4}           # Taylor degree per low-rank head
ACT3_GROUPS = (0, 1, 2)            # k-groups where s3 comes from ACT exp
KGORDER = (3, 0, 1, 2)             # k-group processing order (chain group first)

def monomial_list(maxdeg):
    return [(a, b, d - a - b) for d in range(maxdeg + 1)
            for a in range(d + 1) for b in range(d - a + 1)]

MONS = monomial_list(MAXDEG)
RMAX = len(MONS)                   # 120
RH = {h: len(monomial_list(DEG[h])) for h in (5, 6, 7)}

_cache = {}


def _build(cv):
    key = tuple(cv)
    if key in _cache:
        return _cache[key]
    import concourse.bacc as bacc
    import concourse.tile as tile
    from concourse import mybir

    f32 = mybir.dt.float32
    bf16 = mybir.dt.bfloat16
    AF = mybir.ActivationFunctionType

    nc = bacc.Bacc("TRN2", target_bir_lowering=False, debug=False,
                   num_devices=NCORES)
    ka = nc.dram_tensor("ka", [32, LK], bf16, kind="ExternalInput").ap()
    qa = nc.dram_tensor("qa", [32, QS], bf16, kind="ExternalInput").ap()
    vp = nc.dram_tensor("vp", [128, KT, 5 * V1], bf16, kind="ExternalInput").ap()
    vpf = nc.dram_tensor("vpf", [128, KT, 3 * V1], bf16, kind="ExternalInput").ap()
    psi = nc.dram_tensor("psi", [128, KT, RMAX], bf16, kind="ExternalInput").ap()
    phi = nc.dram_tensor("phi", [RMAX, 3, QS], bf16, kind="ExternalInput").ap()
    wt = nc.dram_tensor("wt", [128, 4, OUTD], bf16, kind="ExternalInput").ap()
    sel8 = nc.dram_tensor("sel8", [8, 4, 128], bf16, kind="ExternalInput").ap()
    outT = nc.dram_tensor("outT", [OUTD, QS], f32, kind="ExternalOutput").ap()

    with tile.TileContext(nc) as tc, ExitStack() as ctx:
        const = ctx.enter_context(tc.tile_pool(name="const", bufs=1))
        spool = ctx.enter_context(tc.tile_pool(name="spool", bufs=16))
        stage = ctx.enter_context(tc.tile_pool(name="stage", bufs=2))
        obuf = ctx.enter_context(tc.tile_pool(name="obuf", bufs=2))
        psp = ctx.enter_context(tc.tile_pool(name="psum", bufs=2, space="PSUM"))

        # inputs on parallel DMA queues: dist operands (sync) land first so
        # the d2 pipeline starts ~immediately; lowrank tables and vp on other
        # queues in parallel.
        # rows 0:15 carry the hi/lo operands, 15:31 host-zeroed; rows 32:128
        # zeroed on-device so only 32 rows transfer but the PE contracts K=128
        # (keeps the HAM activity monitor seeing a dense array).
        ka_sb = const.tile([32, LK], bf16)
        nc.gpsimd.dma_start(out=ka_sb[:], in_=ka)
        qa_sb = const.tile([32, QS], bf16)
        nc.gpsimd.dma_start(out=qa_sb[:], in_=qa)
        psi_sb = const.tile([128, KT, RMAX], bf16)
        nc.gpsimd.dma_start(out=psi_sb[:], in_=psi)
        vpf_sb = const.tile([128, KT, 3 * V1], bf16)
        nc.gpsimd.dma_start(out=vpf_sb[:], in_=vpf)
        vp_sb = const.tile([128, KT, 5 * V1], bf16)
        nc.scalar.dma_start(out=vp_sb[:], in_=vp)
        phi_sb = const.tile([RMAX, 3, QS], bf16)
        nc.gpsimd.dma_start(out=phi_sb[:], in_=phi)
        wt_sb = const.tile([128, 4, OUTD], bf16)
        nc.sync.dma_start(out=wt_sb[:], in_=wt)
        sel8_sb = const.tile([8, 4, 128], bf16)
        nc.sync.dma_start(out=sel8_sb[:], in_=sel8)

        flat = [const.tile([128, QS], bf16, tag=f"flat{j}", name=f"flat{j}")
                for j in range(4)]
        norms = const.tile([8, QS], f32)
        nc.vector.memset(norms[:], 1.0)
        eps_t = const.tile([8, 1], f32)
        nc.vector.memset(eps_t[:], 1e-5)
        r_all = const.tile([8, QS], f32)
        r_hi = const.tile([8, QS], bf16)
        nc.vector.memset(r_hi[:], 0.0)
        r_lo = const.tile([8, QS], bf16)
        nc.vector.memset(r_lo[:], 0.0)
        Mb = const.tile([RMAX, 3 * V1], bf16)

        kt_order = [g * GK + i for g in KGORDER for i in range(GK)]

        # ---- PE: first dist ktiles lead (ACT window starts ASAP), then the
        # lowrank M matmuls fill while the d2 rotation cycles. ----
        kt_lead = kt_order[:2]

        def dist_mms(kt, d2):
            for qc in range(2):
                s5 = slice(qc * 512, (qc + 1) * 512)
                nc.tensor.matmul(d2[:, s5],
                                 lhsT=ka_sb[:, kt * 128:(kt + 1) * 128],
                                 rhs=qa_sb[:, s5], start=True, stop=True)

        d2_lead = {}
        for kt in kt_lead:
            d2_lead[kt] = psp.tile([128, QS], f32, tag="d2", name=f"d2_{kt}")
            dist_mms(kt, d2_lead[kt])
        m_ps = psp.tile([RMAX, 3 * V1], f32, tag="att", name="m_ps")
        for i in range(KT):
            nc.tensor.matmul(m_ps[:], lhsT=psi_sb[:, i, :],
                             rhs=vpf_sb[:, i, :],
                             start=(i == 0), stop=(i == KT - 1))
        nc.vector.tensor_copy(out=Mb[:], in_=m_ps[:])

        latt = {}
        for h in (5, 6, 7):
            a = psp.tile([V1, QS], f32, tag="att", name=f"latt{h}")
            r = RH[h]
            for qc in range(2):
                s5 = slice(qc * 512, (qc + 1) * 512)
                nc.tensor.matmul(a[:, s5],
                                 lhsT=Mb[0:r, (h - 5) * V1:(h - 4) * V1],
                                 rhs=phi_sb[0:r, h - 5, s5],
                                 start=True, stop=True)
            latt[h] = a
        def norm_row(h, a, eng):
            # engines cannot cross to partition h directly (32-align rule):
            # same-partition copy to a stage row, then DMA moves partitions.
            stg = stage.tile([V1, QS], f32, tag="stg", name=f"stg{h}")
            if eng == "v":
                nc.vector.tensor_copy(out=stg[64:65, :], in_=a[64:65, :])
            else:
                nc.scalar.copy(out=stg[64:65, :], in_=a[64:65, :])
            nc.sync.dma_start(out=norms[h:h + 1, :], in_=stg[64:65, :])

        for h in (5, 6, 7):
            j, r0 = h // 2, (h % 2) * 64
            nc.vector.tensor_copy(out=flat[j][r0:r0 + 64, :],
                                  in_=latt[h][0:64, :])
            norm_row(h, latt[h], "v")

        # ---- score tiles ----
        # 20 logical tiles through 16 slots: allocation order pairs the
        # earliest-dying tiles (s1/s4 of the first-processed groups, consumed
        # by in-window attends + chains) with the latest-written ones
        # (idx i reuses slot of idx i-16), so slot reuse never stalls.
        g3, g0, g1, g2 = KGORDER
        s_order = [(1, g3), (4, g3), (1, g0), (4, g0),
                   (1, g1), (1, g2), (4, g1), (4, g2),
                   (3, g3), (3, g0), (3, g1), (3, g2),
                   (0, g3), (0, g0), (2, g3), (2, g0),
                   (2, g2), (0, g2), (2, g1), (0, g1)]
        s_t = {}
        for h, g in s_order:
            s_t[(h, g)] = spool.tile([128, GK, QS], bf16, tag="s",
                                     name=f"s{h}_{g}")

        # ---- in-window accumulators (heads 1 and 4) ----
        att1 = psp.tile([V1, QS], f32, tag="att", name="att1")
        att4 = psp.tile([V1, QS], f32, tag="att", name="att4")

        def attend(h, a, kt, i):
            for qc in range(2):
                s5 = slice(qc * 512, (qc + 1) * 512)
                nc.tensor.matmul(a[:, s5],
                                 lhsT=vp_sb[:, kt, h * V1:(h + 1) * V1],
                                 rhs=s_t[(h, kt // GK)][:, kt % GK, s5],
                                 start=(i == 0), stop=(i == KT - 1))

        # ---- dist + ACT exps + in-window attends, ktile-pipelined ----
        for i, kt in enumerate(kt_order):
            g, kk = kt // GK, kt % GK
            if kt in d2_lead:
                d2 = d2_lead[kt]
            else:
                d2 = psp.tile([128, QS], f32, tag="d2", name=f"d2_{kt}")
                dist_mms(kt, d2)
            nc.scalar.activation(out=s_t[(1, g)][:, kk, :], in_=d2[:],
                                 func=AF.Exp, scale=float(cv[1]))
            nc.scalar.activation(out=s_t[(4, g)][:, kk, :], in_=d2[:],
                                 func=AF.Exp, scale=float(cv[4]))
            if g in ACT3_GROUPS:
                nc.scalar.activation(out=s_t[(3, g)][:, kk, :], in_=d2[:],
                                     func=AF.Exp, scale=float(cv[3]))
            if i >= 2:
                kp = kt_order[i - 2]
                attend(1, att1, kp, i - 2)
                attend(4, att4, kp, i - 2)
        for i in (KT - 2, KT - 1):
            attend(1, att1, kt_order[i], i)
            attend(4, att4, kt_order[i], i)

        # ---- DVE chains, fine-grained (2-ktile slices chase the ACT stream).
        # 2nd mul is in-place (1x DVE mode) -- chains have slack vs the window.
        def chain(dh, sh, g, kk):
            s2k = (slice(None), slice(kk, kk + 2), slice(None))
            nc.vector.tensor_mul(s_t[(dh, g)][s2k], s_t[(sh, g)][s2k],
                                 s_t[(sh, g)][s2k])
            nc.vector.tensor_mul(s_t[(dh, g)][s2k], s_t[(dh, g)][s2k],
                                 s_t[(dh, g)][s2k])

        g3 = KGORDER[0]
        for kk in (0, 2):
            chain(0, 1, g3, kk)
            chain(3, 4, g3, kk)
            chain(2, 3, g3, kk)
        for g in KGORDER[1:]:
            for kk in (0, 2):
                chain(0, 1, g, kk)
                chain(2, 3, g, kk)

        # ---- post-window attends in freed slots (contiguous bursts) ----
        def recip(h0, h1):
            # full-range [0:8] ops (non-32-aligned partition slices are
            # illegal); recomputing other pairs' rows is idempotent and
            # sel8 masks them in the rb matmuls anyway.
            nc.scalar.activation(out=r_all[:], in_=norms[:],
                                 func=AF.Ln, bias=eps_t[:])
            nc.scalar.activation(out=r_all[:], in_=r_all[:],
                                 func=AF.Exp, scale=-1.0)
            nc.vector.tensor_copy(out=r_hi[:], in_=r_all[:])
            nc.vector.tensor_sub(r_lo[:], r_all[:], r_hi[:])

        def rb_mms(rb, j):
            for qc in range(2):
                s5 = slice(qc * 512, (qc + 1) * 512)
                nc.tensor.matmul(rb[:, s5], lhsT=sel8_sb[:, j, :],
                                 rhs=r_hi[:, s5], start=True, stop=False)
                nc.tensor.matmul(rb[:, s5], lhsT=sel8_sb[:, j, :],
                                 rhs=r_lo[:, s5], start=False, stop=True)

        def evac(h, a):
            j, r0 = h // 2, (h % 2) * 64
            nc.vector.tensor_copy(out=flat[j][r0:r0 + 64, :], in_=a[0:64, :])

        def normalize(h, rb):
            j, r0 = h // 2, (h % 2) * 64
            nc.vector.tensor_mul(flat[j][r0:r0 + 64, :],
                                 flat[j][r0:r0 + 64, :], rb[r0:r0 + 64, :])

        # post-window burst order [att2, att3, att0]: the pair-(2,3)
        # normalization ladder then hides under att0's burst, leaving only
        # the (0,1) ladder exposed at the tail.
        att2 = psp.tile([V1, QS], f32, tag="d2", name="att2")
        for i, kt in enumerate(kt_order):
            attend(2, att2, kt, i)
        att3 = psp.tile([V1, QS], f32, tag="d2", name="att3")
        for i, kt in enumerate(kt_order):
            attend(3, att3, kt, i)
        att0 = psp.tile([V1, QS], f32, tag="d2", name="att0")

        # head 1/4 wrap-up (window end)
        norm_row(1, att1, "v")
        evac(1, att1)
        norm_row(4, att4, "v")
        evac(4, att4)

        po = {}

        def po_mms(ot, j, start, stop):
            for qc in range(2):
                s5 = slice(qc * 512, (qc + 1) * 512)
                nc.tensor.matmul(po[ot][:, s5],
                                 lhsT=wt_sb[:, j, ot * 128:(ot + 1) * 128],
                                 rhs=flat[j][:, s5], start=start, stop=stop)

        norm_row(2, att2, "v")
        evac(2, att2)

        # att0 burst
        for i, kt in enumerate(kt_order):
            attend(0, att0, kt, i)

        norm_row(3, att3, "v")
        evac(3, att3)
        recip(2, 8)          # covers pairs (2,3),(4,5),(6,7) under att0

        rb23 = psp.tile([128, QS], f32, tag="att", name="rb23")
        rb_mms(rb23, 1)
        normalize(3, rb23)
        normalize(2, rb23)
        rb45 = psp.tile([128, QS], f32, tag="att", name="rb45")
        rb_mms(rb45, 2)
        normalize(4, rb45)
        normalize(5, rb45)
        rb67 = psp.tile([128, QS], f32, tag="att", name="rb67")
        rb_mms(rb67, 3)
        normalize(6, rb67)
        normalize(7, rb67)

        po[2] = psp.tile([128, QS], f32, tag="d2", name="po2")
        for ji, j in enumerate((1, 2, 3)):
            po_mms(2, j, ji == 0, False)
        po[3] = psp.tile([128, QS], f32, tag="att", name="po3")
        for ji, j in enumerate((1, 2, 3)):
            po_mms(3, j, ji == 0, False)
        po[0] = psp.tile([128, QS], f32, tag="att", name="po0")
        for ji, j in enumerate((1, 2, 3)):
            po_mms(0, j, ji == 0, False)

        # tail ladder for pair (0,1): norm first (it gates everything)
        norm_row(0, att0, "v")
        evac(0, att0)
        recip(0, 2)
        rb01 = psp.tile([128, QS], f32, tag="d2", name="rb01")
        rb_mms(rb01, 0)
        normalize(0, rb01)
        normalize(1, rb01)
        po[1] = psp.tile([128, QS], f32, tag="att", name="po1")
        for ji, j in enumerate((1, 2, 3)):
            po_mms(1, j, ji == 0, False)
        po_mms(2, 0, False, True)
        po_mms(3, 0, False, True)
        po_mms(0, 0, False, True)
        po_mms(1, 0, False, True)

        oq = {0: nc.scalar, 1: nc.gpsimd, 2: nc.sync, 3: nc.gpsimd}
        for ot in range(4):
            ob = obuf.tile([128, QS], f32, tag="ob", name=f"ob{ot}")
            if ot % 2 == 0:
                nc.scalar.copy(out=ob[:], in_=po[ot][:])
            else:
                nc.vector.tensor_copy(out=ob[:], in_=po[ot][:])
            oq[ot].dma_start(out=outT[ot * 128:(ot + 1) * 128, :], in_=ob[:])

    nc.compile()
    _cache[key] = nc
    return nc


def _hilo(x, bf16):
    hi = x.astype(bf16)
    lo = (x - hi.astype(np.float32)).astype(bf16)
    return hi, lo


def _prep_core(qp, kp, vals, mask, w_out, cv, bf16):
    q2 = (qp * qp).sum(-1)
    one_q = np.ones(QS, np.float32)
    qa5 = np.stack([2 * qp[:, 0], 2 * qp[:, 1], 2 * qp[:, 2], -one_q, -q2]) \
        .astype(np.float32)
    k2 = (kp * kp).sum(-1)
    one_k = np.ones(LK, np.float32)
    ka5 = np.stack([kp[:, 0], kp[:, 1], kp[:, 2], k2, one_k]).astype(np.float32)
    ka_hi, ka_lo = _hilo(ka5, bf16)
    qa_hi, qa_lo = _hilo(qa5, bf16)
    # padded to 128 contraction rows (zeros) so the PE array reads as fully
    # active to the HAM clock gate during the dist matmuls.
    ka = np.zeros((32, LK), np.float32).astype(bf16)
    ka[0:5], ka[5:10], ka[10:15] = ka_hi, ka_lo, ka_hi
    qa = np.zeros((32, QS), np.float32).astype(bf16)
    qa[0:5], qa[5:10], qa[10:15] = qa_hi, qa_hi, qa_lo

    vv = np.concatenate([vals, np.ones((LK, H, 1), np.float32)], axis=-1)
    vv = vv.copy()
    vv[mask] = 0.0                               # [LK, H, 65]
    vp = vv[:, 0:5, :].reshape(LK, 5 * V1)
    vp = vp.reshape(KT, 128, 5 * V1).transpose(1, 0, 2).astype(bf16)

    vpf = np.empty((LK, 3, V1), np.float32)
    phi = np.zeros((RMAX, 3, QS), np.float32)
    for h in (5, 6, 7):
        c = float(cv[h])
        f = np.exp(-c * k2)
        vpf[:, h - 5, :] = vv[:, h, :] * f[:, None]
        g = np.exp(-c * q2)
        mons = monomial_list(DEG[h])
        coef = np.array([(2 * c) ** (a + b + cc) /
                         (math.factorial(a) * math.factorial(b) * math.factorial(cc))
                         for (a, b, cc) in mons], np.float32)
        ph = np.stack([qp[:, 0] ** a * qp[:, 1] ** b * qp[:, 2] ** cc
                       for (a, b, cc) in mons], axis=0) * coef[:, None]
        phi[0:len(mons), h - 5, :] = ph * g[None, :]
    vpf = vpf.reshape(KT, 128, 3 * V1).transpose(1, 0, 2).astype(bf16)
    psi = np.stack([kp[:, 0] ** a * kp[:, 1] ** b * kp[:, 2] ** cc
                    for (a, b, cc) in MONS], axis=1)        # [LK, RMAX]
    psi = psi.reshape(KT, 128, RMAX).transpose(1, 0, 2).astype(bf16)

    wt = np.ascontiguousarray(w_out.T).reshape(4, 128, OUTD) \
        .transpose(1, 0, 2).astype(bf16)
    sel8 = np.zeros((8, 4, 128), np.float32)
    for j in range(4):
        sel8[2 * j, j, :64] = 1.0
        sel8[2 * j + 1, j, 64:] = 1.0
    return {"ka": np.ascontiguousarray(ka), "qa": np.ascontiguousarray(qa),
            "vp": np.ascontiguousarray(vp), "vpf": np.ascontiguousarray(vpf),
            "psi": np.ascontiguousarray(psi), "phi": phi.astype(bf16),
            "wt": np.ascontiguousarray(wt), "sel8": sel8.astype(bf16)}


def kernel(query_positions, key_positions, values, masked_elements,
           lengthscales, w_out, _want_trace=False):
    import ml_dtypes
    from concourse.bass_utils import run_bass_kernel_spmd

    bf16 = ml_dtypes.bfloat16
    qp = np.asarray(query_positions, np.float32)
    kp = np.asarray(key_positions, np.float32)
    vals = np.asarray(values, np.float32)
    mask = np.asarray(masked_elements).astype(bool)
    ls = np.asarray(lengthscales, np.float32)
    w = np.asarray(w_out, np.float32)

    cv = (1.0 / (ls.astype(np.float64) ** 2)).astype(np.float32)
    nc = _build(tuple(float(x) for x in cv))

    in_maps = []
    for c in range(NCORES):
        b, hf = c // 2, c % 2
        in_maps.append(_prep_core(qp[b, hf * QS:(hf + 1) * QS], kp[b],
                                  vals[b], mask[b], w, cv, bf16))
    res = run_bass_kernel_spmd(nc, in_maps, core_ids=list(range(NCORES)),
                               trace=_want_trace)
    out = np.empty((B, LQ, OUTD), np.float32)
    for c in range(NCORES):
        b, hf = c // 2, c % 2
        out[b, hf * QS:(hf + 1) * QS, :] = res.results[c]["outT"].T
    if _want_trace:
        return out, res
    return out


# revision 27
# speedup vs baseline: 1.0413x; 1.0135x over previous
"""Trainium2 Bass kernel for KernelAttention (gaussian-kernel multi-head attention).

Math (per batch b):
  d2[q,k]   = |q_pos[q] - k_pos[k]|^2   (computed as m = -d2 via one K=5 augmented matmul)
  s_h[k,q]  = exp(-c_h * d2),  c_h = 1/lengthscale_h^2   (masked keys contribute 0)
  att_h[q,v]= sum_k s_h[k,q] * V[k,h,v] / (sum_k s_h[k,q]*unmasked[k] + 1e-5)
  out[q,o]  = sum_{h,v} att_h[q,v] * w_out[o, h*64+v]

Sharding: 8 cores = (batch b in 0..3) x (query half in 0..1); each core owns
[1024 q, 2048 k]. All inputs host-prepped per core; outputs gathered on host.

Head strategy (c = [100, 25, 4, 1, 0.25, 0.04, 0.01, 0.0025]):
  * heads 5,6,7 (ls=5,10,20): LOW-RANK. exp(-c d2) = g(q) f(k) exp(2c q.k) and
    exp(2c q.k) is a deg<=7 polynomial in q.k => scores factor through <=120
    monomial features psi(k), phi(q). att_h = (psi f V)^T phi' costs O(L*r)
    matmuls; the O(L^2) scores never materialize.
  * heads 1,4 (c=25, 0.25): ACT exp straight from the d2 PSUM per ktile.
  * head 3 (c=1): ACT exp for 3 of 4 k-groups, DVE chain s3=s4^4 for the rest
    (ACT/DVE balance + precision knob).
  * heads 0,2: DVE squaring chains s0=s1^4, s2=s3^4 (bf16, in-place x^2 twice).
PSUM (8 banks) is the binding constraint: tag "d2" = 2 rotating [128,1024]
dist tiles; tag "att" = 2 slots shared in sequence by the lowrank M, lowrank
atts, the two in-window accumulators (heads 1,4), rb broadcasts and po tiles.
Heads 3,0,2 attend post-window in the freed slots. Normalization: ones-column
makes att row 64 the masked score sum; r=1/(n+eps) via ACT Ln/Exp per pair
batch; sel8 matmul broadcasts r (hi/lo bf16) to 128 partitions; DVE applies
it reading the rb PSUM directly.
"""

import math
import numpy as np
from contextlib import ExitStack

B, LQ, LK, DPOS = 4, 2048, 2048, 3
H, V, OUTD = 8, 64, 512
QS = LQ // 2          # q rows per core
KT = LK // 128        # k tiles
NG, GK = 4, KT // 4   # k groups of 4 ktiles
V1 = V + 1            # value cols + ones col
NCORES = 8

MAXDEG = 7
DEG = {5: 7, 6: 5, 7: 4}           # Taylor degree per low-rank head
ACT3_GROUPS = (0, 1, 2)            # k-groups where s3 comes from ACT exp
KGORDER = (3, 0, 1, 2)             # k-group processing order (chain group first)

def monomial_list(maxdeg):
    return [(a, b, d - a - b) for d in range(maxdeg + 1)
            for a in range(d + 1) for b in range(d - a + 1)]

MONS = monomial_list(MAXDEG)
RMAX = len(MONS)                   # 120
RH = {h: len(monomial_list(DEG[h])) for h in (5, 6, 7)}

_cache = {}


def _build(cv):
    key = tuple(cv)
    if key in _cache:
        return _cache[key]
    import concourse.bacc as bacc
    import concourse.tile as tile
    from concourse import mybir

    f32 = mybir.dt.float32
    bf16 = mybir.dt.bfloat16
    AF = mybir.ActivationFunctionType

    nc = bacc.Bacc("TRN2", target_bir_lowering=False, debug=False,
                   num_devices=NCORES)
    ka = nc.dram_tensor("ka", [128, LK], bf16, kind="ExternalInput").ap()
    qa = nc.dram_tensor("qa", [128, QS], bf16, kind="ExternalInput").ap()
    vp = nc.dram_tensor("vp", [128, KT, 5 * V1], bf16, kind="ExternalInput").ap()
    vpf = nc.dram_tensor("vpf", [128, KT, 3 * V1], bf16, kind="ExternalInput").ap()
    psi = nc.dram_tensor("psi", [128, KT, RMAX], bf16, kind="ExternalInput").ap()
    phi = nc.dram_tensor("phi", [RMAX, 3, QS], bf16, kind="ExternalInput").ap()
    wt = nc.dram_tensor("wt", [128, 4, OUTD], bf16, kind="ExternalInput").ap()
    sel8 = nc.dram_tensor("sel8", [8, 4, 128], bf16, kind="ExternalInput").ap()
    outT = nc.dram_tensor("outT", [OUTD, QS], f32, kind="ExternalOutput").ap()

    with tile.TileContext(nc) as tc, ExitStack() as ctx:
        const = ctx.enter_context(tc.tile_pool(name="const", bufs=1))
        spool = ctx.enter_context(tc.tile_pool(name="spool", bufs=16))
        stage = ctx.enter_context(tc.tile_pool(name="stage", bufs=2))
        obuf = ctx.enter_context(tc.tile_pool(name="obuf", bufs=2))
        psp = ctx.enter_context(tc.tile_pool(name="psum", bufs=2, space="PSUM"))

        # inputs on parallel DMA queues: dist operands (sync) land first so
        # the d2 pipeline starts ~immediately; lowrank tables and vp on other
        # queues in parallel.
        ka_sb = const.tile([128, LK], bf16)
        nc.gpsimd.dma_start(out=ka_sb[:], in_=ka)
        qa_sb = const.tile([128, QS], bf16)
        nc.gpsimd.dma_start(out=qa_sb[:], in_=qa)
        psi_sb = const.tile([128, KT, RMAX], bf16)
        nc.gpsimd.dma_start(out=psi_sb[:], in_=psi)
        vpf_sb = const.tile([128, KT, 3 * V1], bf16)
        nc.gpsimd.dma_start(out=vpf_sb[:], in_=vpf)
        vp_sb = const.tile([128, KT, 5 * V1], bf16)
        nc.scalar.dma_start(out=vp_sb[:], in_=vp)
        phi_sb = const.tile([RMAX, 3, QS], bf16)
        nc.gpsimd.dma_start(out=phi_sb[:], in_=phi)
        wt_sb = const.tile([128, 4, OUTD], bf16)
        nc.sync.dma_start(out=wt_sb[:], in_=wt)
        sel8_sb = const.tile([8, 4, 128], bf16)
        nc.sync.dma_start(out=sel8_sb[:], in_=sel8)

        flat = [const.tile([128, QS], bf16, tag=f"flat{j}", name=f"flat{j}")
                for j in range(4)]
        norms = const.tile([8, QS], f32)
        nc.vector.memset(norms[:], 1.0)
        eps_t = const.tile([8, 1], f32)
        nc.vector.memset(eps_t[:], 1e-5)
        r_all = const.tile([8, QS], f32)
        r_hi = const.tile([8, QS], bf16)
        nc.vector.memset(r_hi[:], 0.0)
        r_lo = const.tile([8, QS], bf16)
        nc.vector.memset(r_lo[:], 0.0)
        Mb = const.tile([RMAX, 3 * V1], bf16)

        kt_order = [g * GK + i for g in KGORDER for i in range(GK)]

        # ---- PE: first dist ktiles lead (ACT window starts ASAP), then the
        # lowrank M matmuls fill while the d2 rotation cycles. ----
        kt_lead = kt_order[:2]

        def dist_mms(kt, d2):
            for qc in range(2):
                s5 = slice(qc * 512, (qc + 1) * 512)
                nc.tensor.matmul(d2[:, s5],
                                 lhsT=ka_sb[:, kt * 128:(kt + 1) * 128],
                                 rhs=qa_sb[:, s5], start=True, stop=True)

        d2_lead = {}
        for kt in kt_lead:
            d2_lead[kt] = psp.tile([128, QS], f32, tag="d2", name=f"d2_{kt}")
            dist_mms(kt, d2_lead[kt])
        m_ps = psp.tile([RMAX, 3 * V1], f32, tag="att", name="m_ps")
        for i in range(KT):
            nc.tensor.matmul(m_ps[:], lhsT=psi_sb[:, i, :],
                             rhs=vpf_sb[:, i, :],
                             start=(i == 0), stop=(i == KT - 1))
        nc.vector.tensor_copy(out=Mb[:], in_=m_ps[:])

        latt = {}
        for h in (5, 6, 7):
            a = psp.tile([V1, QS], f32, tag="att", name=f"latt{h}")
            r = RH[h]
            for qc in range(2):
                s5 = slice(qc * 512, (qc + 1) * 512)
                nc.tensor.matmul(a[:, s5],
                                 lhsT=Mb[0:r, (h - 5) * V1:(h - 4) * V1],
                                 rhs=phi_sb[0:r, h - 5, s5],
                                 start=True, stop=True)
            latt[h] = a
        def norm_row(h, a, eng):
            # engines cannot cross to partition h directly (32-align rule):
            # same-partition copy to a stage row, then DMA moves partitions.
            stg = stage.tile([V1, QS], f32, tag="stg", name=f"stg{h}")
            if eng == "v":
                nc.vector.tensor_copy(out=stg[64:65, :], in_=a[64:65, :])
            else:
                nc.scalar.copy(out=stg[64:65, :], in_=a[64:65, :])
            nc.sync.dma_start(out=norms[h:h + 1, :], in_=stg[64:65, :])

        for h in (5, 6, 7):
            j, r0 = h // 2, (h % 2) * 64
            nc.vector.tensor_copy(out=flat[j][r0:r0 + 64, :],
                                  in_=latt[h][0:64, :])
            norm_row(h, latt[h], "v")

        # ---- score tiles ----
        # 20 logical tiles through 16 slots: allocation order pairs the
        # earliest-dying tiles (s1/s4 of the first-processed groups, consumed
        # by in-window attends + chains) with the latest-written ones
        # (idx i reuses slot of idx i-16), so slot reuse never stalls.
        g3, g0, g1, g2 = KGORDER
        s_order = [(1, g3), (4, g3), (1, g0), (4, g0),
                   (1, g1), (1, g2), (4, g1), (4, g2),
                   (3, g3), (3, g0), (3, g1), (3, g2),
                   (0, g3), (0, g0), (2, g3), (2, g0),
                   (2, g2), (0, g2), (2, g1), (0, g1)]
        s_t = {}
        for h, g in s_order:
            s_t[(h, g)] = spool.tile([128, GK, QS], bf16, tag="s",
                                     name=f"s{h}_{g}")

        # ---- in-window accumulators (heads 1 and 4) ----
        att1 = psp.tile([V1, QS], f32, tag="att", name="att1")
        att4 = psp.tile([V1, QS], f32, tag="att", name="att4")

        def attend(h, a, kt, i):
            for qc in range(2):
                s5 = slice(qc * 512, (qc + 1) * 512)
                nc.tensor.matmul(a[:, s5],
                                 lhsT=vp_sb[:, kt, h * V1:(h + 1) * V1],
                                 rhs=s_t[(h, kt // GK)][:, kt % GK, s5],
                                 start=(i == 0), stop=(i == KT - 1))

        # ---- dist + ACT exps + in-window attends, ktile-pipelined ----
        for i, kt in enumerate(kt_order):
            g, kk = kt // GK, kt % GK
            if kt in d2_lead:
                d2 = d2_lead[kt]
            else:
                d2 = psp.tile([128, QS], f32, tag="d2", name=f"d2_{kt}")
                dist_mms(kt, d2)
            nc.scalar.activation(out=s_t[(1, g)][:, kk, :], in_=d2[:],
                                 func=AF.Exp, scale=float(cv[1]))
            nc.scalar.activation(out=s_t[(4, g)][:, kk, :], in_=d2[:],
                                 func=AF.Exp, scale=float(cv[4]))
            if g in ACT3_GROUPS:
                nc.scalar.activation(out=s_t[(3, g)][:, kk, :], in_=d2[:],
                                     func=AF.Exp, scale=float(cv[3]))
            if i >= 2:
                kp = kt_order[i - 2]
                attend(1, att1, kp, i - 2)
                attend(4, att4, kp, i - 2)
        for i in (KT - 2, KT - 1):
            attend(1, att1, kt_order[i], i)
            attend(4, att4, kt_order[i], i)

        # ---- DVE chains, fine-grained (2-ktile slices chase the ACT stream).
        # 2nd mul is in-place (1x DVE mode) -- chains have slack vs the window.
        def chain(dh, sh, g, kk):
            s2k = (slice(None), slice(kk, kk + 2), slice(None))
            nc.vector.tensor_mul(s_t[(dh, g)][s2k], s_t[(sh, g)][s2k],
                                 s_t[(sh, g)][s2k])
            nc.vector.tensor_mul(s_t[(dh, g)][s2k], s_t[(dh, g)][s2k],
                                 s_t[(dh, g)][s2k])

        g3 = KGORDER[0]
        for kk in (0, 2):
            chain(0, 1, g3, kk)
            chain(3, 4, g3, kk)
            chain(2, 3, g3, kk)
        for g in KGORDER[1:]:
            for kk in (0, 2):
                chain(0, 1, g, kk)
                chain(2, 3, g, kk)

        # ---- post-window attends in freed slots (contiguous bursts) ----
        def recip(h0, h1):
            # full-range [0:8] ops (non-32-aligned partition slices are
            # illegal); recomputing other pairs' rows is idempotent and
            # sel8 masks them in the rb matmuls anyway.
            nc.scalar.activation(out=r_all[:], in_=norms[:],
                                 func=AF.Ln, bias=eps_t[:])
            nc.scalar.activation(out=r_all[:], in_=r_all[:],
                                 func=AF.Exp, scale=-1.0)
            nc.vector.tensor_copy(out=r_hi[:], in_=r_all[:])
            nc.vector.tensor_sub(r_lo[:], r_all[:], r_hi[:])

        def rb_mms(rb, j):
            for qc in range(2):
                s5 = slice(qc * 512, (qc + 1) * 512)
                nc.tensor.matmul(rb[:, s5], lhsT=sel8_sb[:, j, :],
                                 rhs=r_hi[:, s5], start=True, stop=False)
                nc.tensor.matmul(rb[:, s5], lhsT=sel8_sb[:, j, :],
                                 rhs=r_lo[:, s5], start=False, stop=True)

        def evac(h, a):
            j, r0 = h // 2, (h % 2) * 64
            nc.vector.tensor_copy(out=flat[j][r0:r0 + 64, :], in_=a[0:64, :])

        def normalize(h, rb):
            j, r0 = h // 2, (h % 2) * 64
            nc.vector.tensor_mul(flat[j][r0:r0 + 64, :],
                                 flat[j][r0:r0 + 64, :], rb[r0:r0 + 64, :])

        # post-window burst order [att2, att3, att0]: the pair-(2,3)
        # normalization ladder then hides under att0's burst, leaving only
        # the (0,1) ladder exposed at the tail.
        att2 = psp.tile([V1, QS], f32, tag="d2", name="att2")
        for i, kt in enumerate(kt_order):
            attend(2, att2, kt, i)
        att3 = psp.tile([V1, QS], f32, tag="d2", name="att3")
        for i, kt in enumerate(kt_order):
            attend(3, att3, kt, i)
        att0 = psp.tile([V1, QS], f32, tag="d2", name="att0")

        # head 1/4 wrap-up (window end)
        norm_row(1, att1, "v")
        evac(1, att1)
        norm_row(4, att4, "v")
        evac(4, att4)

        po = {}

        def po_mms(ot, j, start, stop):
            for qc in range(2):
                s5 = slice(qc * 512, (qc + 1) * 512)
                nc.tensor.matmul(po[ot][:, s5],
                                 lhsT=wt_sb[:, j, ot * 128:(ot + 1) * 128],
                                 rhs=flat[j][:, s5], start=start, stop=stop)

        norm_row(2, att2, "v")
        evac(2, att2)

        # att0 burst
        for i, kt in enumerate(kt_order):
            attend(0, att0, kt, i)

        norm_row(3, att3, "v")
        evac(3, att3)
        recip(2, 8)          # covers pairs (2,3),(4,5),(6,7) under att0

        rb23 = psp.tile([128, QS], f32, tag="att", name="rb23")
        rb_mms(rb23, 1)
        normalize(3, rb23)
        normalize(2, rb23)
        rb45 = psp.tile([128, QS], f32, tag="att", name="rb45")
        rb_mms(rb45, 2)
        normalize(4, rb45)
        normalize(5, rb45)
        rb67 = psp.tile([128, QS], f32, tag="att", name="rb67")
        rb_mms(rb67, 3)
        normalize(6, rb67)
        normalize(7, rb67)

        po[2] = psp.tile([128, QS], f32, tag="d2", name="po2")
        for ji, j in enumerate((1, 2, 3)):
            po_mms(2, j, ji == 0, False)
        po[3] = psp.tile([128, QS], f32, tag="att", name="po3")
        for ji, j in enumerate((1, 2, 3)):
            po_mms(3, j, ji == 0, False)
        po[0] = psp.tile([128, QS], f32, tag="att", name="po0")
        for ji, j in enumerate((1, 2, 3)):
            po_mms(0, j, ji == 0, False)

        # tail ladder for pair (0,1): norm first (it gates everything)
        norm_row(0, att0, "s")
        evac(0, att0)
        recip(0, 2)
        rb01 = psp.tile([128, QS], f32, tag="d2", name="rb01")
        rb_mms(rb01, 0)
        normalize(0, rb01)
        normalize(1, rb01)
        po[1] = psp.tile([128, QS], f32, tag="att", name="po1")
        for ji, j in enumerate((1, 2, 3)):
            po_mms(1, j, ji == 0, False)
        # finish each po and ship it immediately: j0 MMs, evac, DMA per ot
        # (pipelines the 2MB output across the 3 DMA-capable queues).
        oq = {2: nc.sync, 3: nc.gpsimd, 0: nc.scalar, 1: nc.gpsimd}
        for ot in (2, 3, 0, 1):
            po_mms(ot, 0, False, True)
            ob = obuf.tile([128, QS], f32, tag="ob", name=f"ob{ot}")
            if ot % 2 == 0:
                nc.scalar.copy(out=ob[:], in_=po[ot][:])
            else:
                nc.vector.tensor_copy(out=ob[:], in_=po[ot][:])
            oq[ot].dma_start(out=outT[ot * 128:(ot + 1) * 128, :], in_=ob[:])

    nc.compile()
    _cache[key] = nc
    return nc


def _hilo(x, bf16):
    hi = x.astype(bf16)
    lo = (x - hi.astype(np.float32)).astype(bf16)
    return hi, lo


def _prep_core(qp, kp, vals, mask, w_out, cv, bf16):
    q2 = (qp * qp).sum(-1)
    one_q = np.ones(QS, np.float32)
    qa5 = np.stack([2 * qp[:, 0], 2 * qp[:, 1], 2 * qp[:, 2], -one_q, -q2]) \
        .astype(np.float32)
    k2 = (kp * kp).sum(-1)
    one_k = np.ones(LK, np.float32)
    ka5 = np.stack([kp[:, 0], kp[:, 1], kp[:, 2], k2, one_k]).astype(np.float32)
    ka_hi, ka_lo = _hilo(ka5, bf16)
    qa_hi, qa_lo = _hilo(qa5, bf16)
    # padded to 128 contraction rows (zeros) so the PE array reads as fully
    # active to the HAM clock gate during the dist matmuls.
    ka = np.zeros((128, LK), np.float32).astype(bf16)
    ka[0:5], ka[5:10], ka[10:15] = ka_hi, ka_lo, ka_hi
    qa = np.zeros((128, QS), np.float32).astype(bf16)
    qa[0:5], qa[5:10], qa[10:15] = qa_hi, qa_hi, qa_lo

    vv = np.concatenate([vals, np.ones((LK, H, 1), np.float32)], axis=-1)
    vv = vv.copy()
    vv[mask] = 0.0                               # [LK, H, 65]
    vp = vv[:, 0:5, :].reshape(LK, 5 * V1)
    vp = vp.reshape(KT, 128, 5 * V1).transpose(1, 0, 2).astype(bf16)

    vpf = np.empty((LK, 3, V1), np.float32)
    phi = np.zeros((RMAX, 3, QS), np.float32)
    for h in (5, 6, 7):
        c = float(cv[h])
        f = np.exp(-c * k2)
        vpf[:, h - 5, :] = vv[:, h, :] * f[:, None]
        g = np.exp(-c * q2)
        mons = monomial_list(DEG[h])
        coef = np.array([(2 * c) ** (a + b + cc) /
                         (math.factorial(a) * math.factorial(b) * math.factorial(cc))
                         for (a, b, cc) in mons], np.float32)
        ph = np.stack([qp[:, 0] ** a * qp[:, 1] ** b * qp[:, 2] ** cc
                       for (a, b, cc) in mons], axis=0) * coef[:, None]
        phi[0:len(mons), h - 5, :] = ph * g[None, :]
    vpf = vpf.reshape(KT, 128, 3 * V1).transpose(1, 0, 2).astype(bf16)
    psi = np.stack([kp[:, 0] ** a * kp[:, 1] ** b * kp[:, 2] ** cc
                    for (a, b, cc) in MONS], axis=1)        # [LK, RMAX]
    psi = psi.reshape(KT, 128, RMAX).transpose(1, 0, 2).astype(bf16)

    wt = np.ascontiguousarray(w_out.T).reshape(4, 128, OUTD) \
        .transpose(1, 0, 2).astype(bf16)
    sel8 = np.zeros((8, 4, 128), np.float32)
    for j in range(4):
        sel8[2 * j, j, :64] = 1.0
        sel8[2 * j + 1, j, 64:] = 1.0
    return {"ka": np.ascontiguousarray(ka), "qa": np.ascontiguousarray(qa),
            "vp": np.ascontiguousarray(vp), "vpf": np.ascontiguousarray(vpf),
            "psi": np.ascontiguousarray(psi), "phi": phi.astype(bf16),
            "wt": np.ascontiguousarray(wt), "sel8": sel8.astype(bf16)}


def kernel(query_positions, key_positions, values, masked_elements,
           lengthscales, w_out, _want_trace=False):
    import ml_dtypes
    from concourse.bass_utils import run_bass_kernel_spmd

    bf16 = ml_dtypes.bfloat16
    qp = np.asarray(query_positions, np.float32)
    kp = np.asarray(key_positions, np.float32)
    vals = np.asarray(values, np.float32)
    mask = np.asarray(masked_elements).astype(bool)
    ls = np.asarray(lengthscales, np.float32)
    w = np.asarray(w_out, np.float32)

    cv = (1.0 / (ls.astype(np.float64) ** 2)).astype(np.float32)
    nc = _build(tuple(float(x) for x in cv))

    in_maps = []
    for c in range(NCORES):
        b, hf = c // 2, c % 2
        in_maps.append(_prep_core(qp[b, hf * QS:(hf + 1) * QS], kp[b],
                                  vals[b], mask[b], w, cv, bf16))
    res = run_bass_kernel_spmd(nc, in_maps, core_ids=list(range(NCORES)),
                               trace=_want_trace)
    out = np.empty((B, LQ, OUTD), np.float32)
    for c in range(NCORES):
        b, hf = c // 2, c % 2
        out[b, hf * QS:(hf + 1) * QS, :] = res.results[c]["outT"].T
    if _want_trace:
        return out, res
    return out
